# revision 1
# baseline (speedup 1.0000x reference)
"""Trainium2 Bass kernel for nn_MetaLearner (meta-learning attention + cosine
prototype scoring), data-parallel over tasks on 8 NeuronCores.

Math (per task):
  c   = [img, txt] @ Wc.T + bc                (Wc = concat(Wi, Wt))
  h   = LN1(c);  q,k,v = h @ W{q,k,v}.T + b   (queries: seqlen=1 -> ctx = v)
  ctx = softmax(q k^T / sqrt(128)) v          (support: seqlen=4)
  f   = LN2(ctx) @ Wo.T + bo
  logits[t,q,c] = 10 * cos(qf[t,q], sf[t,c])

Host-side folds (all linear, exact):
  - LN gains/biases folded into the following projection weights.
  - LN mean-subtraction folded into the producing weights (column-centered
    weights make the projection output zero-mean; attention outputs of
    centered v stay centered since softmax rows sum to 1).
  - 1/sqrt(128) folded into Wq; the x10 cosine scale into the support norms.
  - Inputs are pre-transposed on host so the contraction dim lands on SBUF
    partitions (f32 DMA-transpose is not available on TRN2's xbar).
On-chip layout is "transposed" throughout: activations are [hid, rows].
Matmul operands use float32r (TF32-like, 1 cyc/row); vector/scalar compute
stays float32.
"""
import sys
sys.path.insert(0, "/opt/trn_rl_repo")
import numpy as np

HID = 128
T, Q, S = 256, 64, 4
DI, DTXT = 2048, 768
NCORES = 8
TPC = T // NCORES               # 32 tasks per core
FEAT = DI + DTXT                # 2816
KT = FEAT // 128                # 22 contraction chunks
QROWS = TPC * Q                 # 2048 query rows per core
SROWS = TPC * S                 # 128 support rows per core
ROWS = QROWS + SROWS            # 2176
CHUNKS = [(0, 512), (512, 512), (1024, 512), (1536, 512)]  # query chunks
SCALE_INV = 1.0 / (np.sqrt(HID) + 1e-8)
EPS = 1e-5

_prog = None  # cached compiled Bass program


def _build():
    import concourse.bacc as bacc
    import concourse.tile as tile
    import concourse.mybir as mybir
    import concourse.bass as _b

    F32 = mybir.dt.float32
    F32R = mybir.dt.float32r
    AFT = mybir.ActivationFunctionType
    AX = mybir.AxisListType
    ALU = mybir.AluOpType

    nc = bacc.Bacc()
    xT_d = nc.declare_dram_parameter("xT", [FEAT, ROWS], F32R, isOutput=False)
    wc_d = nc.declare_dram_parameter("wc", [FEAT, HID], F32R, isOutput=False)
    wqkvo_d = nc.declare_dram_parameter("wqkvo", [HID, 4 * HID], F32R,
                                        isOutput=False)
    bias_d = nc.declare_dram_parameter("biases", [HID, 6], F32, isOutput=False)
    onesr_d = nc.declare_dram_parameter("onesr", [1, HID], F32R, isOutput=False)
    mask_d = nc.declare_dram_parameter("mask", [SROWS, SROWS], F32, isOutput=False)
    id_d = nc.declare_dram_parameter("ident", [128, 128], F32, isOutput=False)
    out_d = nc.declare_dram_parameter("logits", [TPC, Q, S], F32, isOutput=True)

    lp = nc.allow_low_precision(reason="float32r tiles are bit-compatible f32")
    lp.__enter__()

    ACHUNKS = CHUNKS + [(QROWS, SROWS)]   # 4 query chunks + support chunk

    with tile.TileContext(nc) as tc:
        with (
            tc.tile_pool(name="wts", bufs=1) as wts,
            tc.tile_pool(name="qfp", bufs=1) as qfp,
            tc.tile_pool(name="xp", bufs=3) as xp,
            tc.tile_pool(name="wk", bufs=2) as wk,
            tc.tile_pool(name="ps", bufs=1, space="PSUM") as ps,
        ):
            # ---- stream-critical loads first (SP issues in program order) ----
            wc_t = wts.tile([128, KT, HID], F32R)
            nc.sync.dma_start(out=wc_t[:, 0, :], in_=wc_d[0:128, :])
            xa_tiles = []

            def _xa(k):
                x_t = xp.tile([128, 640], F32R, tag="xa", name=f"xa{k}")
                nc.sync.dma_start(out=x_t,
                                  in_=xT_d[k * 128:(k + 1) * 128, 0:640])
                xa_tiles.append(x_t)

            def _wc(k0, k1):
                nc.sync.dma_start(
                    out=wc_t[:, k0:k1, :],
                    in_=wc_d[k0 * 128:k1 * 128].rearrange(
                        "(c p) m -> p c m", p=128))

            _xa(0)
            _wc(1, 3)
            _xa(1)
            _wc(3, 8)
            _xa(2)
            _wc(8, KT)
            wqkvo_t = wts.tile([128, 4 * HID], F32R)
            nc.sync.dma_start(out=wqkvo_t, in_=wqkvo_d[:])
            wq_t = wqkvo_t[:, 0 * HID:1 * HID]
            wk_t = wqkvo_t[:, 1 * HID:2 * HID]
            wv_t = wqkvo_t[:, 2 * HID:3 * HID]
            wo_t = wqkvo_t[:, 3 * HID:4 * HID]
            bias_t = wts.tile([HID, 6], F32)
            nc.sync.dma_start(out=bias_t, in_=bias_d[:])
            bc_t = bias_t[:, 0:1]
            bq_t = bias_t[:, 1:2]
            bk_t = bias_t[:, 2:3]
            bv_t = bias_t[:, 3:4]
            bo_t = bias_t[:, 4:5]
            ones_c = wts.tile([128, 1], F32R)    # ss-reduce lhsT [K=128, M=1]
            nc.gpsimd.dma_start(out=ones_c, in_=bias_d[:, 5:6])
            ones_r = wts.tile([1, 128], F32R)    # broadcast lhsT [K=1, M=128]
            nc.sync.dma_start(out=ones_r, in_=onesr_d[:])
            eps_t = wts.tile([1, 1], F32)
            nc.vector.memset(eps_t, EPS)

            qf_tiles = [qfp.tile([128, 512], F32R, tag=f"qf{n}", name=f"qf{n}")
                        for n in range(4)]
            sf_t = qfp.tile([128, SROWS], F32R, tag="sft")

            def rstd_bcast_sb(psum, src_sb, cn, scale):
                """SBUF f32 src -> 1/sqrt(mean(src^2)+eps) bcast [128,cn] PSUM."""
                sq = wk.tile([128, 512], F32R, tag="sq")
                nc.scalar.activation(out=sq[:, :cn], in_=src_sb[:, :cn],
                                     func=AFT.Square, bias=0.0, scale=1.0)
                ss_ps = psum.tile([1, 512], F32, tag="ssps", bufs=2)
                nc.tensor.matmul(ss_ps[:, :cn], ones_c[:], sq[:, :cn],
                                 start=True, stop=True)
                var_r = wk.tile([1, 512], F32, tag="var")
                nc.scalar.activation(out=var_r[:, :cn], in_=ss_ps[:, :cn],
                                     func=AFT.Sqrt, bias=eps_t[:], scale=scale)
                rstd_r = wk.tile([1, 512], F32R, tag="rstdr")
                nc.vector.reciprocal(out=rstd_r[:, :cn], in_=var_r[:, :cn])
                R_ps = psum.tile([128, 512], F32, tag="rps", bufs=2)
                nc.tensor.matmul(R_ps[:, :cn], ones_r[:], rstd_r[:, :cn],
                                 start=True, stop=True)
                return R_ps

            def rstd_bcast_ps(psum, src_ps, bias, cn, scale):
                """PSUM src (+bias) -> rstd bcast [128,cn] PSUM."""
                sq = wk.tile([128, 512], F32R, tag="sq2")
                nc.scalar.activation(out=sq[:, :cn], in_=src_ps[:, :cn],
                                     func=AFT.Square, bias=bias, scale=1.0)
                ss_ps = psum.tile([1, 512], F32, tag="ssps", bufs=2)
                nc.tensor.matmul(ss_ps[:, :cn], ones_c[:], sq[:, :cn],
                                 start=True, stop=True)
                var_r = wk.tile([1, 512], F32, tag="var")
                nc.scalar.activation(out=var_r[:, :cn], in_=ss_ps[:, :cn],
                                     func=AFT.Sqrt, bias=eps_t[:], scale=scale)
                rstd_r = wk.tile([1, 512], F32R, tag="rstdr")
                nc.vector.reciprocal(out=rstd_r[:, :cn], in_=var_r[:, :cn])
                R_ps = psum.tile([128, 512], F32, tag="rps", bufs=2)
                nc.tensor.matmul(R_ps[:, :cn], ones_r[:], rstd_r[:, :cn],
                                 start=True, stop=True)
                return R_ps

            def col_normalize(psum, dst, src_ps, bias, cn, sqs, clip):
                """dst = (src+bias) / max(||col||*sqs, clip) per column."""
                sq = wk.tile([128, 512], F32R, tag="sq3")
                nc.scalar.activation(out=sq[:, :cn], in_=src_ps[:, :cn],
                                     func=AFT.Square, bias=bias, scale=1.0)
                ss_ps = psum.tile([1, 512], F32, tag="ssps", bufs=2)
                nc.tensor.matmul(ss_ps[:, :cn], ones_c[:], sq[:, :cn],
                                 start=True, stop=True)
                n_r = wk.tile([1, 512], F32, tag="nrm")
                nc.scalar.activation(out=n_r[:, :cn], in_=ss_ps[:, :cn],
                                     func=AFT.Sqrt, bias=0.0, scale=sqs)
                nc.vector.tensor_scalar_max(out=n_r[:, :cn], in0=n_r[:, :cn],
                                            scalar1=clip)
                i_r = wk.tile([1, 512], F32R, tag="inrm")
                nc.vector.reciprocal(out=i_r[:, :cn], in_=n_r[:, :cn])
                I_ps = psum.tile([128, 512], F32, tag="rps", bufs=2)
                nc.tensor.matmul(I_ps[:, :cn], ones_r[:], i_r[:, :cn],
                                 start=True, stop=True)
                raw = wk.tile([128, 512], F32, tag="qraw")
                nc.scalar.activation(out=raw[:, :cn], in_=src_ps[:, :cn],
                                     func=AFT.Identity, bias=bias, scale=1.0)
                nc.vector.tensor_mul(out=dst[:, :cn], in0=raw[:, :cn],
                                     in1=I_ps[:, :cn])

            def query_tail(pst, qf_tile, qf_off, c_f, cn):
                R1 = rstd_bcast_sb(pst, c_f, cn, 1.0 / HID)
                h_t = wk.tile([128, 512], F32R, tag="h")
                nc.vector.tensor_mul(out=h_t[:, :cn], in0=c_f[:, :cn],
                                     in1=R1[:, :cn])
                v_ps = pst.tile([128, 512], F32, tag="pps", bufs=2)
                nc.tensor.matmul(v_ps[:, :cn], wv_t, h_t[:, :cn],
                                 start=True, stop=True)
                R2 = rstd_bcast_ps(pst, v_ps, bv_t, cn, 1.0 / HID)
                v_f = wk.tile([128, 512], F32, tag="vf")
                nc.scalar.activation(out=v_f[:, :cn], in_=v_ps[:, :cn],
                                     func=AFT.Identity, bias=bv_t, scale=1.0)
                z_t = wk.tile([128, 512], F32R, tag="z")
                nc.vector.tensor_mul(out=z_t[:, :cn], in0=v_f[:, :cn],
                                     in1=R2[:, :cn])
                o_ps = pst.tile([128, 512], F32, tag="pps", bufs=2)
                nc.tensor.matmul(o_ps[:, :cn], wo_t, z_t[:, :cn],
                                 start=True, stop=True)
                dst = qf_tile[:, qf_off:qf_off + cn]
                col_normalize(pst, dst, o_ps, bo_t, cn, 1.0, 1e-8)

            # streaming groups, ascending completion time; every tail except
            # the last group's hides under a later group's DMA stream
            # (xT col layout: [support | q0 | q1 | q2 | q3])
            G1 = [("s", 0, SROWS), (0, SROWS, 512)]
            G2 = [(1, 640, 512), (2, 1152, 512)]
            G3 = [("3a", 1664, 256), ("3b", 1920, 256)]
            c_fs = {}

            with tc.tile_pool(name="pst", bufs=1, space="PSUM") as pst:
                # ---- group 1 ----
                with tc.tile_pool(name="ps1", bufs=1, space="PSUM") as ps1:
                    c_pss = {n: ps1.tile([128, cn], F32, tag=f"cps{n}",
                                         name=f"cps_{n}")
                             for (n, c0, cn) in G1}
                    for k in range(KT):
                        if k < 3:
                            x_t = xa_tiles[k]
                        else:
                            x_t = xp.tile([128, 640], F32R, tag="xa")
                            nc.sync.dma_start(
                                out=x_t, in_=xT_d[k * 128:(k + 1) * 128, 0:640])
                        for (n, c0, cn) in G1:
                            nc.tensor.matmul(c_pss[n][:, :cn], wc_t[:, k, :],
                                             x_t[:, c0:c0 + cn],
                                             start=(k == 0), stop=(k == KT - 1))
                    for (n, c0, cn) in G1:
                        c_f = wk.tile([128, 512], F32, tag=f"cf{n}",
                                      name=f"cf{n}")
                        nc.scalar.activation(out=c_f[:, :cn],
                                             in_=c_pss[n][:, :cn],
                                             func=AFT.Identity, bias=bc_t,
                                             scale=1.0)
                        c_fs[n] = c_f

                mask_t = wts.tile([SROWS, SROWS], F32)
                nc.sync.dma_start(out=mask_t, in_=mask_d[:])
                id_t = wts.tile([128, 128], F32)
                nc.sync.dma_start(out=id_t, in_=id_d[:])

                # ---- group 2 streams; support + q0 tails run underneath ----
                with tc.tile_pool(name="ps2", bufs=1, space="PSUM") as ps2:
                    c_pss = {n: ps2.tile([128, cn], F32, tag=f"cps{n}",
                                         name=f"cps_{n}")
                             for (n, c0, cn) in G2}
                    for k in range(KT):
                        x_t = xp.tile([128, 1024], F32R, tag="xb")
                        nc.sync.dma_start(
                            out=x_t, in_=xT_d[k * 128:(k + 1) * 128, 640:1664])
                        for (n, c0, cn) in G2:
                            nc.tensor.matmul(c_pss[n][:, :cn], wc_t[:, k, :],
                                             x_t[:, c0 - 640:c0 - 640 + cn],
                                             start=(k == 0), stop=(k == KT - 1))

                    # support tail
                    cn = SROWS
                    cs_f = c_fs["s"]
                    R1 = rstd_bcast_sb(pst, cs_f, cn, 1.0 / HID)
                    h_t = wk.tile([128, SROWS], F32R, tag="sh")
                    nc.vector.tensor_mul(out=h_t, in0=cs_f[:, :cn],
                                         in1=R1[:, :cn])
                    q_ps = pst.tile([128, SROWS], F32, tag="pps", bufs=2)
                    nc.tensor.matmul(q_ps[:], wq_t, h_t[:], start=True, stop=True)
                    qT = wk.tile([128, SROWS], F32R, tag="qT")
                    nc.scalar.activation(out=qT, in_=q_ps, func=AFT.Identity,
                                         bias=bq_t, scale=1.0)
                    k_ps = pst.tile([128, SROWS], F32, tag="pps", bufs=2)
                    nc.tensor.matmul(k_ps[:], wk_t, h_t[:], start=True, stop=True)
                    kT = wk.tile([128, SROWS], F32R, tag="kT")
                    nc.scalar.activation(out=kT, in_=k_ps, func=AFT.Identity,
                                         bias=bk_t, scale=1.0)
                    v_ps = pst.tile([128, SROWS], F32, tag="pps", bufs=2)
                    nc.tensor.matmul(v_ps[:], wv_t, h_t[:], start=True, stop=True)
                    vT_f = wk.tile([128, SROWS], F32, tag="vTf")
                    nc.scalar.activation(out=vT_f, in_=v_ps, func=AFT.Identity,
                                         bias=bv_t, scale=1.0)

                    s_ps = pst.tile([SROWS, SROWS], F32, tag="rps", bufs=2)
                    nc.tensor.matmul(s_ps[:], qT[:], kT[:], start=True, stop=True)
                    s_f = wk.tile([SROWS, SROWS], F32, tag="sf_")
                    nc.vector.tensor_add(out=s_f, in0=s_ps, in1=mask_t)
                    nmx = wk.tile([SROWS, 1], F32, tag="nmx")
                    nc.vector.tensor_reduce(out=nmx, in_=s_f, axis=AX.X,
                                            op=ALU.max, negate=True)
                    a_f = wk.tile([SROWS, SROWS], F32, tag="af")
                    asum = wk.tile([SROWS, 1], F32, tag="asum")
                    nc.scalar.activation(out=a_f, in_=s_f, func=AFT.Exp,
                                         bias=nmx, scale=1.0, accum_out=asum)
                    rs = wk.tile([SROWS, 1], F32, tag="rs")
                    nc.vector.reciprocal(out=rs, in_=asum)
                    nc.vector.tensor_scalar_mul(out=a_f, in0=a_f, scalar1=rs)

                    aT_ps = pst.tile([SROWS, SROWS], F32, tag="pps", bufs=2)
                    nc.tensor.matmul(aT_ps[:], a_f[:], id_t[:], is_transpose=True)
                    aT = wk.tile([SROWS, SROWS], F32, tag="aT")
                    nc.vector.tensor_copy(out=aT, in_=aT_ps)
                    vn_ps = pst.tile([SROWS, SROWS], F32, tag="pps", bufs=2)
                    nc.tensor.matmul(vn_ps[:], vT_f[:], id_t[:], is_transpose=True)
                    vn = wk.tile([SROWS, SROWS], F32, tag="vn")
                    nc.vector.tensor_copy(out=vn, in_=vn_ps)
                    ctx_ps = pst.tile([128, SROWS], F32, tag="pps", bufs=2)
                    nc.tensor.matmul(ctx_ps[:], vn[:], aT[:], start=True, stop=True)
                    ctx_f = wk.tile([128, SROWS], F32, tag="ctxf")
                    nc.scalar.activation(out=ctx_f, in_=ctx_ps, func=AFT.Copy,
                                         scale=1.0)

                    R2 = rstd_bcast_sb(pst, ctx_f, cn, 1.0 / HID)
                    z_t = wk.tile([128, SROWS], F32R, tag="sz")
                    nc.vector.tensor_mul(out=z_t, in0=ctx_f, in1=R2[:, :cn])
                    o_ps = pst.tile([128, SROWS], F32, tag="pps", bufs=2)
                    nc.tensor.matmul(o_ps[:], wo_t, z_t[:], start=True, stop=True)
                    col_normalize(pst, sf_t, o_ps, bo_t, cn, 0.01, 1e-9)

                    # q0 tail
                    query_tail(pst, qf_tiles[0], 0, c_fs[0], 512)

                    for (n, c0, cn) in G2:
                        c_f = wk.tile([128, 512], F32, tag=f"cf{n}",
                                      name=f"cf{n}")
                        nc.scalar.activation(out=c_f[:, :cn],
                                             in_=c_pss[n][:, :cn],
                                             func=AFT.Identity, bias=bc_t,
                                             scale=1.0)
                        c_fs[n] = c_f

                # ---- group 3 streams; q1/q2 tails underneath ----
                with tc.tile_pool(name="ps3", bufs=1, space="PSUM") as ps3:
                    c_pss = {n: ps3.tile([128, cn], F32, tag=f"cps{n}",
                                         name=f"cps_{n}")
                             for (n, c0, cn) in G3}
                    for k in range(KT):
                        x_t = xp.tile([128, 512], F32R, tag="xc")
                        nc.sync.dma_start(
                            out=x_t, in_=xT_d[k * 128:(k + 1) * 128, 1664:ROWS])
                        for (n, c0, cn) in G3:
                            nc.tensor.matmul(c_pss[n][:, :cn], wc_t[:, k, :],
                                             x_t[:, c0 - 1664:c0 - 1664 + cn],
                                             start=(k == 0), stop=(k == KT - 1))

                    query_tail(pst, qf_tiles[1], 0, c_fs[1], 512)
                    query_tail(pst, qf_tiles[2], 0, c_fs[2], 512)

                    for (n, c0, cn) in G3:
                        c_f = wk.tile([128, 512], F32, tag=f"cf{n}",
                                      name=f"cf{n}")
                        nc.scalar.activation(out=c_f[:, :cn],
                                             in_=c_pss[n][:, :cn],
                                             func=AFT.Identity, bias=bc_t,
                                             scale=1.0)
                        c_fs[n] = c_f

                # ---- last tails (half-width, short chains) ----
                query_tail(pst, qf_tiles[3], 0, c_fs["3a"], 256)
                query_tail(pst, qf_tiles[3], 256, c_fs["3b"], 256)

                # ---- per-task scores + output ----
                with tc.tile_pool(name="psu", bufs=1, space="PSUM") as psu:
                    U_ps = psu.tile([64, 2 * Q], F32, tag="ups", name="U_ps")
                    for t in range(TPC):
                        g = t // 2
                        nc.tensor.matmul(
                            U_ps[0:64,
                                 64 * (t % 2) + 4 * g:64 * (t % 2) + 4 * g + 4],
                            qf_tiles[t // 8][:, 64 * (t % 8):64 * (t % 8) + 64],
                            sf_t[:, 4 * t:4 * t + 4],
                            start=True, stop=True)
                    out_base = out_d[:]
                    for half in range(2):
                        U_sb = wk.tile([64, Q], F32, tag=f"usb{half}",
                                       name=f"usb{half}")
                        nc.vector.tensor_copy(
                            out=U_sb, in_=U_ps[0:64, 64 * half:64 * half + 64])
                        dst = _b.AP(tensor=out_base.tensor,
                                    offset=out_base.offset + 256 * half,
                                    ap=[[4, 64], [512, 16], [1, 4]])
                        nc.sync.dma_start(
                            out=dst, in_=U_sb.rearrange("p (g b) -> p g b", b=4))

    lp.__exit__(None, None, None)
    nc.compile()
    return nc


def _host_prep(inputs):
    f32 = np.float32
    Wi, Wt = np.asarray(inputs["Wi"], f32), np.asarray(inputs["Wt"], f32)
    bi, bt = np.asarray(inputs["bi"], f32), np.asarray(inputs["bt"], f32)
    g1, b1 = np.asarray(inputs["g1"], f32), np.asarray(inputs["b1"], f32)
    g2, b2 = np.asarray(inputs["g2"], f32), np.asarray(inputs["b2"], f32)
    Wq, bq = np.asarray(inputs["Wq"], f32), np.asarray(inputs["bq"], f32)
    Wk, bk = np.asarray(inputs["Wk"], f32), np.asarray(inputs["bk"], f32)
    Wv, bv = np.asarray(inputs["Wv"], f32), np.asarray(inputs["bv"], f32)
    Wo, bo = np.asarray(inputs["Wo"], f32), np.asarray(inputs["bo"], f32)

    Wc = np.concatenate([Wi, Wt], axis=1)          # [128, 2816]
    bc = bi + bt
    Wc_c = Wc - Wc.mean(axis=0, keepdims=True)     # fold LN1 mean
    bc_c = bc - bc.mean()

    Wq_f = (Wq * g1[None, :]) * SCALE_INV
    bq_f = (bq + Wq @ b1) * SCALE_INV
    Wk_f = Wk * g1[None, :]
    bk_f = bk + Wk @ b1
    Wv_f = Wv * g1[None, :]
    bv_f = bv + Wv @ b1
    Wv_c = Wv_f - Wv_f.mean(axis=0, keepdims=True)  # fold LN2 mean
    bv_c = bv_f - bv_f.mean()
    Wo_f = Wo * g2[None, :]
    bo_f = bo + Wo @ b2

    blk = np.arange(SROWS) // S
    mask = np.where(blk[:, None] == blk[None, :], 0.0, -1e30).astype(f32)

    wqkvo = np.concatenate([Wq_f.T, Wk_f.T, Wv_c.T, Wo_f.T], axis=1)
    biases = np.stack([bc_c, bq_f, bk_f, bv_c, bo_f,
                       np.ones(HID, f32)], axis=1)
    common = {
        "wc": np.ascontiguousarray(Wc_c.T),
        "wqkvo": np.ascontiguousarray(wqkvo),
        "biases": np.ascontiguousarray(biases),
        "onesr": np.ones((1, HID), f32),
        "mask": mask, "ident": np.eye(128, dtype=f32),
    }

    si = np.asarray(inputs["support_images"], f32)
    st = np.asarray(inputs["support_texts"], f32)
    qi = np.asarray(inputs["query_images"], f32)
    qt = np.asarray(inputs["query_texts"], f32)

    in_maps = []
    for m in range(NCORES):
        ts = slice(m * TPC, (m + 1) * TPC)
        Xq = np.concatenate([qi[ts].reshape(QROWS, DI),
                             qt[ts].reshape(QROWS, DTXT)], axis=1)
        Xs = np.concatenate([si[ts].reshape(SROWS, DI),
                             st[ts].reshape(SROWS, DTXT)], axis=1)
        X = np.concatenate([Xs, Xq], axis=0)        # [2176, 2816] support first
        xT = np.ascontiguousarray(X.T)              # [2816, 2176]
        in_maps.append({"xT": xT, **common})
    return in_maps


def _run(in_maps, trace=False, **kw):
    from concourse.bass_utils import run_bass_kernel_spmd
    global _prog
    if _prog is None:
        _prog = _build()
    return run_bass_kernel_spmd(_prog, in_maps, list(range(NCORES)),
                                trace=trace, **kw)


def kernel(**inputs) -> np.ndarray:
    in_maps = _host_prep(inputs)
    res = _run(in_maps)
    return np.concatenate([res.results[m]["logits"] for m in range(NCORES)],
                          axis=0)



# revision 10
# speedup vs baseline: 1.9589x; 1.9589x over previous
"""Trainium2 Bass kernel for nn_MetaLearner (meta-learning attention + cosine
prototype scoring), data-parallel over tasks on 8 NeuronCores.

Math (per task):
  c   = [img, txt] @ Wc.T + bc                (Wc = concat(Wi, Wt))
  h   = LN1(c);  q,k,v = h @ W{q,k,v}.T + b   (queries: seqlen=1 -> ctx = v)
  ctx = softmax(q k^T / sqrt(128)) v          (support: seqlen=4)
  f   = LN2(ctx) @ Wo.T + bo
  logits[t,q,c] = 10 * cos(qf[t,q], sf[t,c])

Key folds (all exact for this problem's parameters):
  - LN gains/biases folded into following projections; LN mean-subtraction
    folded into producing weights (column-centered weights give zero-mean
    projections, preserved through attention since softmax rows sum to 1).
  - With column-centered weights, LN is a pure per-column scale; the final
    cosine normalization is scale-invariant, so for the QUERY path (seqlen 1,
    zero bv/bo) the entire tail collapses:
        colnorm(Wo.LN2(Wv.LN1(c))) == colnorm((Wo@Wv@Wc) . x)
    The composed weights stream through one GEMM; only a column-normalize
    remains per query block.  The support path keeps full attention, but its
    LN2 is likewise absorbed by the cosine normalization.
  - 1/sqrt(128) folded into Wq; the x10 cosine scale into the support norms.
  - rsqrt computed as exp(-0.5*ln(x)) so every scalar-engine function
    (square/identity/copy/exp/ln) lives in ONE activation table set
    (natural_log_exp_and_others): exactly one ACT_TABLE_LOAD, no switches.
  - Activations stream as float16 (halves HBM traffic vs f32); on-chip
    accumulation in f32 PSUM; support-tail compute in f32/f32r.
"""
import sys
sys.path.insert(0, "/opt/trn_rl_repo")
import numpy as np

HID = 128
T, Q, S = 256, 64, 4
DI, DTXT = 2048, 768
NCORES = 8
TPC = T // NCORES               # 32 tasks per core
FEAT = DI + DTXT                # 2816
KT = FEAT // 128                # 22 contraction chunks
QROWS = TPC * Q                 # 2048 query rows per core
SROWS = TPC * S                 # 128 support rows per core
ROWS = QROWS + SROWS            # 2176
P1W = SROWS + 1024              # pass-1 columns: support + q0 + q1
P2W = 1024                      # pass-2 columns: q2 + q3
SCALE_INV = 1.0 / (np.sqrt(HID) + 1e-8)
EPS = 1e-5

_prog = None  # cached compiled Bass program


def _build():
    import concourse.bacc as bacc
    import concourse.tile as tile
    import concourse.mybir as mybir
    import concourse.bass as _b

    F32 = mybir.dt.float32
    F32R = mybir.dt.float32r
    F16 = mybir.dt.float16
    AFT = mybir.ActivationFunctionType
    AX = mybir.AxisListType
    ALU = mybir.AluOpType

    nc = bacc.Bacc()
    xT_d = nc.declare_dram_parameter("xT", [FEAT, ROWS], F16, isOutput=False)
    ws_d = nc.declare_dram_parameter("ws", [FEAT, HID], F16, isOutput=False)
    wqs_d = nc.declare_dram_parameter("wqs", [FEAT, HID], F16, isOutput=False)
    wqkvo_d = nc.declare_dram_parameter("wqkvo", [HID, 4 * HID], F32R,
                                        isOutput=False)
    bias_d = nc.declare_dram_parameter("biases", [HID, 8], F32, isOutput=False)
    onesr_d = nc.declare_dram_parameter("onesr", [1, HID], F32R, isOutput=False)
    mask_d = nc.declare_dram_parameter("mask", [SROWS, SROWS], F32, isOutput=False)
    id_d = nc.declare_dram_parameter("ident", [128, 128], F32, isOutput=False)
    out_d = nc.declare_dram_parameter("logits", [TPC, Q, S], F32, isOutput=True)

    lp = nc.allow_low_precision(reason="f16 stream / f32r tail matmuls")
    lp.__enter__()

    with tile.TileContext(nc) as tc:
        with (
            tc.tile_pool(name="wts", bufs=1) as wts,
            tc.tile_pool(name="qfp", bufs=1) as qfp,
            tc.tile_pool(name="xap", bufs=KT) as xap,
            tc.tile_pool(name="xbp", bufs=KT) as xbp,
            tc.tile_pool(name="wk", bufs=2) as wk,
        ):
            ws_t = wts.tile([128, KT, HID], F16)
            wqs_t = wts.tile([128, KT, HID], F16)

            def _w(dst, src_d, k0, k1):
                nc.scalar.dma_start(
                    out=dst[:, k0:k1, :],
                    in_=src_d[k0 * 128:k1 * 128].rearrange(
                        "(c p) m -> p c m", p=128))

            xa_tiles, xb_tiles = [], []

            def _xa(k):
                t = xap.tile([128, P1W], F16, tag="xa", name=f"xa{k}")
                eng = nc.sync if k % 2 == 0 else nc.gpsimd
                eng.dma_start(out=t, in_=xT_d[k * 128:(k + 1) * 128, 0:P1W])
                xa_tiles.append(t)

            def _xb(k):
                t = xbp.tile([128, P2W], F16, tag="xb", name=f"xb{k}")
                eng = nc.sync if k % 2 == 0 else nc.gpsimd
                eng.dma_start(out=t, in_=xT_d[k * 128:(k + 1) * 128, P1W:ROWS])
                xb_tiles.append(t)

            # prelude: weights on the scalar queue, x stream on sync+vector
            _w(ws_t, ws_d, 0, 2)
            _xa(0)
            _w(wqs_t, wqs_d, 0, 2)
            _xa(1)
            _w(ws_t, ws_d, 2, 12)
            _xa(2)
            _w(wqs_t, wqs_d, 2, 12)
            _xa(3)
            _w(ws_t, ws_d, 12, KT)
            _xa(4)
            _w(wqs_t, wqs_d, 12, KT)
            for k in range(5, KT):
                _xa(k)
            wqkvo_t = wts.tile([128, 4 * HID], F32R)
            nc.scalar.dma_start(out=wqkvo_t, in_=wqkvo_d[:])
            wq_t = wqkvo_t[:, 0 * HID:1 * HID]
            wk_t = wqkvo_t[:, 1 * HID:2 * HID]
            wv_t = wqkvo_t[:, 2 * HID:3 * HID]
            wo_t = wqkvo_t[:, 3 * HID:4 * HID]
            bias_t = wts.tile([HID, 8], F32)
            nc.scalar.dma_start(out=bias_t, in_=bias_d[:])
            bc_t = bias_t[:, 0:1]
            bq_t = bias_t[:, 1:2]
            bk_t = bias_t[:, 2:3]
            ub_t = bias_t[:, 3:4]
            ones_c = wts.tile([128, 1], F32R)    # ss-reduce lhsT [K=128, M=1]
            nc.gpsimd.dma_start(out=ones_c, in_=bias_d[:, 4:5])  # f32->f32r cast
            ones_r = wts.tile([1, 128], F32R)    # broadcast lhsT [K=1, M=128]
            nc.scalar.dma_start(out=ones_r, in_=onesr_d[:])
            mask_t = wts.tile([SROWS, SROWS], F32)
            nc.scalar.dma_start(out=mask_t, in_=mask_d[:])
            id_t = wts.tile([128, 128], F32)
            nc.scalar.dma_start(out=id_t, in_=id_d[:])
            eps_t = wts.tile([1, 1], F32)
            nc.vector.memset(eps_t, EPS)
            tiny_t = wts.tile([1, 1], F32)
            nc.vector.memset(tiny_t, 1e-30)
            for k in range(KT):
                _xb(k)

            qf_tiles = [qfp.tile([128, 512], F16, tag=f"qf{j}", name=f"qf{j}")
                        for j in range(4)]
            sf_t = qfp.tile([128, SROWS], F16, tag="sft")

            with tc.tile_pool(name="pu", bufs=1, space="PSUM") as pu:
                u_ts = [pu.tile([128, 512], F32, tag=f"u{j}", name=f"u{j}")
                        for j in range(4)]

                # ---- pass 1: support + q0 + q1 columns ----
                with tc.tile_pool(name="psA", bufs=1, space="PSUM") as psA:
                    c_s = psA.tile([128, SROWS], F32, tag="cs")
                    for k in range(KT):
                        x_t = xa_tiles[k]
                        st = dict(start=(k == 0), stop=(k == KT - 1))
                        nc.tensor.matmul(c_s[:], ws_t[:, k, :],
                                         x_t[:, 0:SROWS], **st)
                        nc.tensor.matmul(u_ts[0][:], wqs_t[:, k, :],
                                         x_t[:, SROWS:SROWS + 512], **st)
                        nc.tensor.matmul(u_ts[1][:], wqs_t[:, k, :],
                                         x_t[:, SROWS + 512:P1W], **st)
                    cs_f = wk.tile([128, SROWS], F32, tag="csf")
                    nc.scalar.activation(out=cs_f, in_=c_s, func=AFT.Identity,
                                         bias=bc_t, scale=1.0)

                # ---- pass 2a ----
                for k in range(0, 12):
                    x_t = xb_tiles[k]
                    st = dict(start=(k == 0), stop=(k == KT - 1))
                    nc.tensor.matmul(u_ts[2][:], wqs_t[:, k, :],
                                     x_t[:, 0:512], **st)
                    nc.tensor.matmul(u_ts[3][:], wqs_t[:, k, :],
                                     x_t[:, 512:P2W], **st)

                with tc.tile_pool(name="pst", bufs=1, space="PSUM") as pst:

                    def rstd_bcast(sq_src, n, scale, eps):
                        """sq_src: squared values [128, n] -> bcast
                        exp(-0.5*ln(scale*sum+eps)) as [128, n] PSUM."""
                        ss = pst.tile([1, 512], F32, tag="ssps", bufs=1)
                        nc.tensor.matmul(ss[:, :n], ones_c[:], sq_src[:, :n],
                                         start=True, stop=True)
                        ln_r = wk.tile([1, 512], F32, tag="lnr")
                        nc.scalar.activation(out=ln_r[:, :n], in_=ss[:, :n],
                                             func=AFT.Ln, bias=eps[:], scale=scale)
                        ir = wk.tile([1, 512], F32R, tag="rsr")
                        nc.scalar.activation(out=ir[:, :n], in_=ln_r[:, :n],
                                             func=AFT.Exp, bias=0.0, scale=-0.5)
                        R = pst.tile([128, 512], F32, tag="rps", bufs=1)
                        nc.tensor.matmul(R[:, :n], ones_r[:], ir[:, :n],
                                         start=True, stop=True)
                        return R

                    def qcolnorm(u_ps, dst):
                        """dst = (u+ub) / ||u+ub|| per column, f16 out."""
                        raw = wk.tile([128, 512], F32, tag="raw")
                        nc.vector.tensor_scalar_add(out=raw, in0=u_ps,
                                                    scalar1=ub_t)
                        sq = wk.tile([128, 512], F32R, tag="sq")
                        nc.vector.tensor_mul(out=sq, in0=raw, in1=raw)
                        R = rstd_bcast(sq, 512, 1.0, tiny_t)
                        nc.vector.tensor_mul(out=dst[:], in0=raw, in1=R[:, 0:512])

                    # ---- support tail (full attention on 128 columns) ----
                    sqs_ = wk.tile([128, SROWS], F32R, tag="sqs")
                    nc.scalar.activation(out=sqs_, in_=cs_f, func=AFT.Square,
                                         bias=0.0, scale=1.0)
                    R1 = rstd_bcast(sqs_, SROWS, 1.0 / HID, eps_t)
                    h_t = wk.tile([128, SROWS], F32R, tag="h")
                    nc.vector.tensor_mul(out=h_t, in0=cs_f, in1=R1[:, :SROWS])
                    q_ps = pst.tile([128, SROWS], F32, tag="pps", bufs=1)
                    nc.tensor.matmul(q_ps[:], wq_t, h_t[:], start=True, stop=True)
                    qT = wk.tile([128, SROWS], F32R, tag="qT")
                    nc.scalar.activation(out=qT, in_=q_ps, func=AFT.Identity,
                                         bias=bq_t, scale=1.0)
                    k_ps = pst.tile([128, SROWS], F32, tag="pps", bufs=1)
                    nc.tensor.matmul(k_ps[:], wk_t, h_t[:], start=True, stop=True)
                    kT = wk.tile([128, SROWS], F32R, tag="kT")
                    nc.scalar.activation(out=kT, in_=k_ps, func=AFT.Identity,
                                         bias=bk_t, scale=1.0)
                    # v in natural [rows, hid] layout: lhsT=h (K=hid, M=rows)
                    vn_ps = pst.tile([128, SROWS], F32, tag="pps", bufs=1)
                    nc.tensor.matmul(vn_ps[:], h_t[:], wv_t, start=True, stop=True)
                    vn = wk.tile([SROWS, HID], F32R, tag="vn")
                    nc.vector.tensor_copy(out=vn, in_=vn_ps)

                    s_ps = pst.tile([SROWS, SROWS], F32, tag="pps", bufs=1)
                    nc.tensor.matmul(s_ps[:], qT[:], kT[:], start=True, stop=True)
                    s_f = wk.tile([SROWS, SROWS], F32, tag="sf_")
                    nc.vector.tensor_add(out=s_f, in0=s_ps, in1=mask_t)
                    nmx = wk.tile([SROWS, 1], F32, tag="nmx")
                    nc.vector.tensor_reduce(out=nmx, in_=s_f, axis=AX.X,
                                            op=ALU.max, negate=True)
                    a_f = wk.tile([SROWS, SROWS], F32, tag="af")
                    asum = wk.tile([SROWS, 1], F32, tag="asum")
                    nc.scalar.activation(out=a_f, in_=s_f, func=AFT.Exp,
                                         bias=nmx, scale=1.0, accum_out=asum)
                    rs = wk.tile([SROWS, 1], F32, tag="rs")
                    nc.vector.reciprocal(out=rs, in_=asum)
                    nc.vector.tensor_scalar_mul(out=a_f, in0=a_f, scalar1=rs)

                    aT_ps = pst.tile([SROWS, SROWS], F32, tag="pps", bufs=1)
                    nc.tensor.matmul(aT_ps[:], a_f[:], id_t[:], is_transpose=True)
                    aT = wk.tile([SROWS, SROWS], F32R, tag="aT")
                    nc.vector.tensor_copy(out=aT, in_=aT_ps)
                    ctx_ps = pst.tile([128, SROWS], F32, tag="pps", bufs=1)
                    nc.tensor.matmul(ctx_ps[:], vn[:], aT[:], start=True, stop=True)
                    ctx_f = wk.tile([128, SROWS], F32R, tag="ctxf")
                    nc.vector.tensor_copy(out=ctx_f, in_=ctx_ps)
                    o_ps = pst.tile([128, SROWS], F32, tag="pps", bufs=1)
                    nc.tensor.matmul(o_ps[:], wo_t, ctx_f[:], start=True, stop=True)
                    # colnorm with x10 folded: 1/sqrt(0.01*ss) = 10/||o||
                    raws = wk.tile([128, SROWS], F32, tag="raws")
                    nc.vector.tensor_copy(out=raws, in_=o_ps)
                    sq2 = wk.tile([128, SROWS], F32R, tag="sqs")
                    nc.scalar.activation(out=sq2, in_=o_ps, func=AFT.Square,
                                         bias=0.0, scale=1.0)
                    R2 = rstd_bcast(sq2, SROWS, 0.01, tiny_t)
                    nc.vector.tensor_mul(out=sf_t[:], in0=raws,
                                         in1=R2[:, :SROWS])

                    # q0/q1 column-normalize (overlap pass-2 stream)
                    qcolnorm(u_ts[0], qf_tiles[0])
                    qcolnorm(u_ts[1], qf_tiles[1])

                    # ---- pass 2b ----
                    for k in range(12, 18):
                        x_t = xb_tiles[k]
                        st = dict(start=False, stop=(k == KT - 1))
                        nc.tensor.matmul(u_ts[2][:], wqs_t[:, k, :],
                                         x_t[:, 0:512], **st)
                        nc.tensor.matmul(u_ts[3][:], wqs_t[:, k, :],
                                         x_t[:, 512:P2W], **st)

                    # ---- scores for tasks 0..15 ----
                    out_base = out_d[:]

                    def score_half(hf):
                        U = pst.tile([64, 64], F32, tag="ups", bufs=1,
                                     name=f"ups{hf}")
                        for i in range(16):
                            t = 16 * hf + i
                            j = t // 8
                            col = 64 * (t % 8)
                            nc.tensor.matmul(
                                U[0:64, 4 * i:4 * i + 4],
                                qf_tiles[j][:, col:col + 64],
                                sf_t[:, 4 * t:4 * t + 4],
                                start=True, stop=True)
                        U_sb = wk.tile([64, 64], F32, tag="usb",
                                       name=f"usb{hf}")
                        nc.vector.tensor_copy(out=U_sb, in_=U)
                        dst = _b.AP(tensor=out_base.tensor,
                                    offset=out_base.offset + 4096 * hf,
                                    ap=[[4, 64], [256, 16], [1, 4]])
                        nc.sync.dma_start(
                            out=dst, in_=U_sb.rearrange("p (g b) -> p g b", b=4))

                    score_half(0)

                    # ---- pass 2c ----
                    for k in range(18, KT):
                        x_t = xb_tiles[k]
                        st = dict(start=False, stop=(k == KT - 1))
                        nc.tensor.matmul(u_ts[2][:], wqs_t[:, k, :],
                                         x_t[:, 0:512], **st)
                        nc.tensor.matmul(u_ts[3][:], wqs_t[:, k, :],
                                         x_t[:, 512:P2W], **st)

                    qcolnorm(u_ts[2], qf_tiles[2])
                    qcolnorm(u_ts[3], qf_tiles[3])
                    score_half(1)

    lp.__exit__(None, None, None)
    nc.compile()
    return nc


def _host_prep(inputs):
    f32 = np.float32
    Wi, Wt = np.asarray(inputs["Wi"], f32), np.asarray(inputs["Wt"], f32)
    bi, bt = np.asarray(inputs["bi"], f32), np.asarray(inputs["bt"], f32)
    g1, b1 = np.asarray(inputs["g1"], f32), np.asarray(inputs["b1"], f32)
    g2, b2 = np.asarray(inputs["g2"], f32), np.asarray(inputs["b2"], f32)
    Wq, bq = np.asarray(inputs["Wq"], f32), np.asarray(inputs["bq"], f32)
    Wk, bk = np.asarray(inputs["Wk"], f32), np.asarray(inputs["bk"], f32)
    Wv, bv = np.asarray(inputs["Wv"], f32), np.asarray(inputs["bv"], f32)
    Wo, bo = np.asarray(inputs["Wo"], f32), np.asarray(inputs["bo"], f32)

    Wc = np.concatenate([Wi, Wt], axis=1)          # [128, 2816]
    bc = bi + bt
    Wc_c = Wc - Wc.mean(axis=0, keepdims=True)     # fold LN1 mean
    bc_c = bc - bc.mean()

    Wq_f = (Wq * g1[None, :]) * SCALE_INV
    bq_f = (bq + Wq @ b1) * SCALE_INV
    Wk_f = Wk * g1[None, :]
    bk_f = bk + Wk @ b1
    Wv_f = Wv * g1[None, :]
    bv_f = bv + Wv @ b1
    Wv_c = Wv_f - Wv_f.mean(axis=0, keepdims=True)  # fold LN2 mean
    bv_c = bv_f - bv_f.mean()
    Wo_f = Wo * g2[None, :]
    bo_f = bo + Wo @ b2

    # fast path requires the value/output biases to vanish (true for this
    # problem: all reference biases are zeros) -- the cosine normalization
    # then absorbs both LN scales on the query path and LN2 on support.
    assert np.abs(bv_c).max() < 1e-6 and np.abs(bo_f).max() < 1e-6, \
        "nonzero bv/bo: collapsed query path would be inexact"

    Wov = Wo_f @ Wv_c                              # [128, 128]
    Wqs = Wov @ Wc_c                               # [128, 2816]
    ub = Wov @ bc_c                                # [128]

    blk = np.arange(SROWS) // S
    mask = np.where(blk[:, None] == blk[None, :], 0.0, -1e30).astype(f32)

    wqkvo = np.concatenate([Wq_f.T, Wk_f.T, Wv_c.T, Wo_f.T], axis=1)
    biases = np.stack([bc_c, bq_f, bk_f, ub, np.ones(HID, f32),
                       np.zeros(HID, f32), np.zeros(HID, f32),
                       np.zeros(HID, f32)], axis=1)
    common = {
        "ws": np.ascontiguousarray(Wc_c.T).astype(np.float16),
        "wqs": np.ascontiguousarray(Wqs.T).astype(np.float16),
        "wqkvo": np.ascontiguousarray(wqkvo),
        "biases": np.ascontiguousarray(biases),
        "onesr": np.ones((1, HID), f32),
        "mask": mask, "ident": np.eye(128, dtype=f32),
    }

    si = np.asarray(inputs["support_images"], f32)
    st = np.asarray(inputs["support_texts"], f32)
    qi = np.asarray(inputs["query_images"], f32)
    qt = np.asarray(inputs["query_texts"], f32)

    in_maps = []
    for m in range(NCORES):
        ts = slice(m * TPC, (m + 1) * TPC)
        Xq = np.concatenate([qi[ts].reshape(QROWS, DI),
                             qt[ts].reshape(QROWS, DTXT)], axis=1)
        Xs = np.concatenate([si[ts].reshape(SROWS, DI),
                             st[ts].reshape(SROWS, DTXT)], axis=1)
        X = np.concatenate([Xs, Xq], axis=0)        # [2176, 2816] support first
        xT = np.ascontiguousarray(X.T.astype(np.float16))  # [2816, 2176] f16
        in_maps.append({"xT": xT, **common})
    return in_maps


def _run(in_maps, trace=False, **kw):
    from concourse.bass_utils import run_bass_kernel_spmd
    global _prog
    if _prog is None:
        _prog = _build()
    return run_bass_kernel_spmd(_prog, in_maps, list(range(NCORES)),
                                trace=trace, **kw)


def kernel(**inputs) -> np.ndarray:
    in_maps = _host_prep(inputs)
    res = _run(in_maps)
    return np.concatenate([res.results[m]["logits"] for m in range(NCORES)],
                          axis=0)


# revision 12
# speedup vs baseline: 2.5210x; 1.2870x over previous
"""Trainium2 Bass kernel for nn_MetaLearner (meta-learning attention + cosine
prototype scoring), data-parallel over tasks on 8 NeuronCores.

Math (per task):
  c   = [img, txt] @ Wc.T + bc                (Wc = concat(Wi, Wt))
  h   = LN1(c);  q,k,v = h @ W{q,k,v}.T + b   (queries: seqlen=1 -> ctx = v)
  ctx = softmax(q k^T / sqrt(128)) v          (support: seqlen=4)
  f   = LN2(ctx) @ Wo.T + bo
  logits[t,q,c] = 10 * cos(qf[t,q], sf[t,c])

Key folds (exact for this problem's parameters; asserted on host):
  - LN gains/biases folded into following projections; LN mean-subtraction
    folded into producing weights (column-centered weights give zero-mean
    projections, preserved through attention since softmax rows sum to 1).
  - With centered weights LN is a pure per-column scale and cosine similarity
    is scale-invariant, so the QUERY path (seqlen 1, zero bv/bo) collapses:
        cos(Wo.LN2(Wv.LN1(c)), p) == cos((Wo@Wv@Wc).x, p)
    One composed-weight GEMM streams all query work; support keeps full
    attention but its LN2 is likewise absorbed.
  - The cosine DIVISIONS happen on the host: the device ships raw dot
    products U = qf_raw^T sf_raw plus squared norms (ones-vector matmuls,
    DMA'd straight from PSUM); kernel() divides after the gather.  No
    rsqrt/reciprocal runs on device, so the scalar engine needs only
    {identity, square, ln, exp, copy}: 3 activation-table loads total.
  - 1/sqrt(128) folded into Wq.  Inputs stream as float16 (halves HBM
    traffic); f32 PSUM accumulation; support tail in f32/f32r.
"""
import sys
sys.path.insert(0, "/opt/trn_rl_repo")
import numpy as np

HID = 128
T, Q, S = 256, 64, 4
DI, DTXT = 2048, 768
NCORES = 8
TPC = T // NCORES               # 32 tasks per core
FEAT = DI + DTXT                # 2816
KT = FEAT // 128                # 22 contraction chunks
QROWS = TPC * Q                 # 2048 query rows per core
SROWS = TPC * S                 # 128 support rows per core
ROWS = QROWS + SROWS            # 2176
P1W = SROWS + 1024              # pass-1 columns: support + q0 + q1
P2W = 1024                      # pass-2 columns: q2 + q3
SCALE_INV = 1.0 / (np.sqrt(HID) + 1e-8)
EPS = 1e-5

_prog = None  # cached compiled Bass program


def _build():
    import concourse.bacc as bacc
    import concourse.tile as tile
    import concourse.mybir as mybir
    import concourse.bass as _b

    F32 = mybir.dt.float32
    F32R = mybir.dt.float32r
    F16 = mybir.dt.float16
    AFT = mybir.ActivationFunctionType
    AX = mybir.AxisListType
    ALU = mybir.AluOpType

    nc = bacc.Bacc()
    xT_d = nc.declare_dram_parameter("xT", [FEAT, ROWS], F16, isOutput=False)
    ws_d = nc.declare_dram_parameter("ws", [128, KT * HID], F16, isOutput=False)
    wqs_d = nc.declare_dram_parameter("wqs", [128, KT * HID], F16,
                                      isOutput=False)
    wqkvo_d = nc.declare_dram_parameter("wqkvo", [HID, 4 * HID], F32R,
                                        isOutput=False)
    bias_d = nc.declare_dram_parameter("biases", [HID, 8], F32, isOutput=False)
    onesr_d = nc.declare_dram_parameter("onesr", [1, HID], F32R, isOutput=False)
    mask_d = nc.declare_dram_parameter("mask", [SROWS, SROWS], F32, isOutput=False)
    id_d = nc.declare_dram_parameter("ident", [128, 128], F32, isOutput=False)
    u_d = nc.declare_dram_parameter("uraw", [TPC, Q, S], F32, isOutput=True)
    ssq_d = nc.declare_dram_parameter("ssq", [1, QROWS], F32, isOutput=True)
    sss_d = nc.declare_dram_parameter("sss", [1, SROWS], F32, isOutput=True)

    lp = nc.allow_low_precision(reason="f16 stream / f32r tail matmuls")
    lp.__enter__()

    with tile.TileContext(nc) as tc:
        with (
            tc.tile_pool(name="wts", bufs=1) as wts,
            tc.tile_pool(name="qfp", bufs=1) as qfp,
            tc.tile_pool(name="xap", bufs=KT) as xap,
            tc.tile_pool(name="xbp", bufs=KT) as xbp,
            tc.tile_pool(name="wk", bufs=2) as wk,
        ):
            ws_t = wts.tile([128, KT * HID], F16)
            wqs_t = wts.tile([128, KT * HID], F16)
            wqkvo_t = wts.tile([128, 4 * HID], F32R)
            bias_t = wts.tile([HID, 8], F32)
            ones_c = wts.tile([128, 1], F32R)
            ones_r = wts.tile([1, 128], F32R)
            mask_t = wts.tile([SROWS, SROWS], F32)
            id_t = wts.tile([128, 128], F32)
            eps_t = wts.tile([1, 1], F32)

            xa_tiles, xb_tiles = [], []

            def _xa(k):
                t = xap.tile([128, P1W], F16, tag="xa", name=f"xa{k}")
                eng = nc.sync if k % 2 == 0 else nc.gpsimd
                eng.dma_start(out=t, in_=xT_d[k * 128:(k + 1) * 128, 0:P1W])
                xa_tiles.append(t)

            def _xb(k):
                t = xbp.tile([128, P2W], F16, tag="xb", name=f"xb{k}")
                eng = nc.sync if k % 2 == 0 else nc.gpsimd
                eng.dma_start(out=t, in_=xT_d[k * 128:(k + 1) * 128, P1W:ROWS])
                xb_tiles.append(t)

            # prelude: weights first (pre-chunked 2D layout -> one cheap DMA
            # each), x stream split across the sync and gpsimd queues
            nc.sync.dma_start(out=ws_t, in_=ws_d[:])
            nc.gpsimd.dma_start(out=wqs_t, in_=wqs_d[:])
            nc.gpsimd.dma_start(out=ones_c, in_=bias_d[:, 4:5])  # f32r cast
            _xa(0)
            _xa(1)
            nc.sync.dma_start(out=wqkvo_t, in_=wqkvo_d[:])
            _xa(2)
            _xa(3)
            nc.sync.dma_start(out=bias_t, in_=bias_d[:])
            nc.sync.dma_start(out=ones_r, in_=onesr_d[:])
            _xa(4)
            _xa(5)
            nc.sync.dma_start(out=mask_t, in_=mask_d[:])
            nc.sync.dma_start(out=id_t, in_=id_d[:])
            for k in range(6, KT):
                _xa(k)
            for k in range(KT):
                _xb(k)
            nc.vector.memset(eps_t, EPS)

            bc_t = bias_t[:, 0:1]
            bq_t = bias_t[:, 1:2]
            bk_t = bias_t[:, 2:3]
            ub_t = bias_t[:, 3:4]
            wq_t = wqkvo_t[:, 0 * HID:1 * HID]
            wk_t = wqkvo_t[:, 1 * HID:2 * HID]
            wv_t = wqkvo_t[:, 2 * HID:3 * HID]
            wo_t = wqkvo_t[:, 3 * HID:4 * HID]

            # raw (unnormalized) features, f32r for the scoring matmuls
            qraw = [qfp.tile([128, 512], F32R, tag=f"qr{j}", name=f"qr{j}")
                    for j in range(4)]
            sraw = qfp.tile([128, SROWS], F32R, tag="sr")
            ssq_sb = qfp.tile([1, QROWS], F32, tag="ssqsb")
            sss_sb = qfp.tile([1, SROWS], F32, tag="ssssb")

            with tc.tile_pool(name="pu", bufs=1, space="PSUM") as pu:
                u_ts = [pu.tile([128, 512], F32, tag=f"u{j}", name=f"u{j}")
                        for j in range(4)]

                # ---- pass 1: support + q0 + q1 columns ----
                with tc.tile_pool(name="psA", bufs=1, space="PSUM") as psA:
                    c_s = psA.tile([128, SROWS], F32, tag="cs")
                    for k in range(KT):
                        x_t = xa_tiles[k]
                        st = dict(start=(k == 0), stop=(k == KT - 1))
                        nc.tensor.matmul(c_s[:], ws_t[:, k * HID:(k + 1) * HID],
                                         x_t[:, 0:SROWS], **st)
                        nc.tensor.matmul(u_ts[0][:],
                                         wqs_t[:, k * HID:(k + 1) * HID],
                                         x_t[:, SROWS:SROWS + 512], **st)
                        nc.tensor.matmul(u_ts[1][:],
                                         wqs_t[:, k * HID:(k + 1) * HID],
                                         x_t[:, SROWS + 512:P1W], **st)
                    cs_f = wk.tile([128, SROWS], F32, tag="csf")
                    nc.scalar.activation(out=cs_f, in_=c_s, func=AFT.Identity,
                                         bias=bc_t, scale=1.0)

                # ---- pass 2a ----
                for k in range(0, 12):
                    x_t = xb_tiles[k]
                    st = dict(start=(k == 0), stop=(k == KT - 1))
                    nc.tensor.matmul(u_ts[2][:],
                                     wqs_t[:, k * HID:(k + 1) * HID],
                                     x_t[:, 0:512], **st)
                    nc.tensor.matmul(u_ts[3][:],
                                     wqs_t[:, k * HID:(k + 1) * HID],
                                     x_t[:, 512:P2W], **st)

                with tc.tile_pool(name="pst", bufs=1, space="PSUM") as pst:

                    def qnorms(j):
                        """raw_j = u_j + ub; ship ||col||^2 straight to DRAM."""
                        nc.vector.tensor_scalar_add(out=qraw[j], in0=u_ts[j],
                                                    scalar1=ub_t)
                        sq = wk.tile([128, 512], F32R, tag="sq")
                        nc.vector.tensor_mul(out=sq, in0=qraw[j], in1=qraw[j])
                        ss = pst.tile([1, 512], F32, tag="ssps", bufs=1)
                        nc.tensor.matmul(ss[:], ones_c[:], sq[:],
                                         start=True, stop=True)
                        nc.scalar.activation(
                            out=ssq_sb[:, 512 * j:512 * (j + 1)], in_=ss[:],
                            func=AFT.Copy, scale=1.0)

                    # ---- support tail (full attention on 128 columns) ----
                    sqs_ = wk.tile([128, SROWS], F32R, tag="sqs")
                    nc.scalar.activation(out=sqs_, in_=cs_f, func=AFT.Square,
                                         bias=0.0, scale=1.0)
                    ss1 = pst.tile([1, 512], F32, tag="ssps", bufs=1)
                    nc.tensor.matmul(ss1[:, :SROWS], ones_c[:], sqs_[:],
                                     start=True, stop=True)
                    ln_r = wk.tile([1, SROWS], F32, tag="lnr")
                    nc.scalar.activation(out=ln_r, in_=ss1[:, :SROWS],
                                         func=AFT.Ln, bias=eps_t[:],
                                         scale=1.0 / HID)
                    ir = wk.tile([1, SROWS], F32R, tag="rsr")
                    nc.scalar.activation(out=ir, in_=ln_r, func=AFT.Exp,
                                         bias=0.0, scale=-0.5)
                    R1 = pst.tile([128, SROWS], F32, tag="rps", bufs=1)
                    nc.tensor.matmul(R1[:], ones_r[:], ir[:],
                                     start=True, stop=True)
                    h_t = wk.tile([128, SROWS], F32R, tag="h")
                    nc.vector.tensor_mul(out=h_t, in0=cs_f, in1=R1[:])

                    q_ps = pst.tile([128, SROWS], F32, tag="pps", bufs=1)
                    nc.tensor.matmul(q_ps[:], wq_t, h_t[:], start=True, stop=True)
                    qT = wk.tile([128, SROWS], F32R, tag="qT")
                    nc.scalar.activation(out=qT, in_=q_ps, func=AFT.Identity,
                                         bias=bq_t, scale=1.0)
                    k_ps = pst.tile([128, SROWS], F32, tag="pps", bufs=1)
                    nc.tensor.matmul(k_ps[:], wk_t, h_t[:], start=True, stop=True)
                    kT = wk.tile([128, SROWS], F32R, tag="kT")
                    nc.scalar.activation(out=kT, in_=k_ps, func=AFT.Identity,
                                         bias=bk_t, scale=1.0)
                    # v in natural [rows, hid] layout: lhsT=h (K=hid, M=rows)
                    vn_ps = pst.tile([128, SROWS], F32, tag="pps", bufs=1)
                    nc.tensor.matmul(vn_ps[:], h_t[:], wv_t, start=True, stop=True)
                    vn = wk.tile([SROWS, HID], F32R, tag="vn")
                    nc.vector.tensor_copy(out=vn, in_=vn_ps)

                    s_ps = pst.tile([SROWS, SROWS], F32, tag="pps", bufs=1)
                    nc.tensor.matmul(s_ps[:], qT[:], kT[:], start=True, stop=True)
                    s_f = wk.tile([SROWS, SROWS], F32, tag="sf_")
                    nc.vector.tensor_add(out=s_f, in0=s_ps, in1=mask_t)
                    nmx = wk.tile([SROWS, 1], F32, tag="nmx")
                    nc.vector.tensor_reduce(out=nmx, in_=s_f, axis=AX.X,
                                            op=ALU.max, negate=True)
                    a_f = wk.tile([SROWS, SROWS], F32, tag="af")
                    asum = wk.tile([SROWS, 1], F32, tag="asum")
                    nc.scalar.activation(out=a_f, in_=s_f, func=AFT.Exp,
                                         bias=nmx, scale=1.0, accum_out=asum)
                    rs = wk.tile([SROWS, 1], F32, tag="rs")
                    nc.vector.reciprocal(out=rs, in_=asum)
                    nc.vector.tensor_scalar_mul(out=a_f, in0=a_f, scalar1=rs)

                    aT_ps = pst.tile([SROWS, SROWS], F32, tag="pps", bufs=1)
                    nc.tensor.matmul(aT_ps[:], a_f[:], id_t[:], is_transpose=True)
                    aT = wk.tile([SROWS, SROWS], F32R, tag="aT")
                    nc.vector.tensor_copy(out=aT, in_=aT_ps)
                    ctx_ps = pst.tile([128, SROWS], F32, tag="pps", bufs=1)
                    nc.tensor.matmul(ctx_ps[:], vn[:], aT[:], start=True, stop=True)
                    ctx_f = wk.tile([128, SROWS], F32R, tag="ctxf")
                    nc.vector.tensor_copy(out=ctx_f, in_=ctx_ps)
                    o_ps = pst.tile([128, SROWS], F32, tag="pps", bufs=1)
                    nc.tensor.matmul(o_ps[:], wo_t, ctx_f[:], start=True, stop=True)
                    nc.scalar.activation(out=sraw, in_=o_ps, func=AFT.Copy,
                                         scale=1.0)
                    sqo = wk.tile([128, SROWS], F32R, tag="sqo")
                    nc.vector.tensor_mul(out=sqo, in0=sraw, in1=sraw)
                    ss2 = pst.tile([1, 512], F32, tag="ssps", bufs=1)
                    nc.tensor.matmul(ss2[:, :SROWS], ones_c[:], sqo[:],
                                     start=True, stop=True)
                    nc.scalar.activation(out=sss_sb[:], in_=ss2[:, :SROWS],
                                         func=AFT.Copy, scale=1.0)
                    nc.gpsimd.dma_start(out=sss_d[:], in_=sss_sb[:])

                    # q0/q1 raw features + norms (overlap pass-2 stream)
                    qnorms(0)
                    qnorms(1)

                    # ---- pass 2b ----
                    for k in range(12, 18):
                        x_t = xb_tiles[k]
                        st = dict(start=False, stop=(k == KT - 1))
                        nc.tensor.matmul(u_ts[2][:],
                                         wqs_t[:, k * HID:(k + 1) * HID],
                                         x_t[:, 0:512], **st)
                        nc.tensor.matmul(u_ts[3][:],
                                         wqs_t[:, k * HID:(k + 1) * HID],
                                         x_t[:, 512:P2W], **st)

                    u_base = u_d[:]

                    def score_half(hf):
                        U = pst.tile([64, 64], F32, tag="ups", bufs=1,
                                     name=f"ups{hf}")
                        for i in range(16):
                            t = 16 * hf + i
                            j = t // 8
                            col = 64 * (t % 8)
                            nc.tensor.matmul(
                                U[0:64, 4 * i:4 * i + 4],
                                qraw[j][:, col:col + 64],
                                sraw[:, 4 * t:4 * t + 4],
                                start=True, stop=True)
                        U_sb = wk.tile([64, 64], F32, tag="usb",
                                       name=f"usb{hf}")
                        nc.vector.tensor_copy(out=U_sb, in_=U)
                        dst = _b.AP(tensor=u_base.tensor,
                                    offset=u_base.offset + 4096 * hf,
                                    ap=[[4, 64], [256, 16], [1, 4]])
                        nc.sync.dma_start(
                            out=dst, in_=U_sb.rearrange("p (g b) -> p g b", b=4))

                    score_half(0)

                    # ---- pass 2c ----
                    for k in range(18, KT):
                        x_t = xb_tiles[k]
                        st = dict(start=False, stop=(k == KT - 1))
                        nc.tensor.matmul(u_ts[2][:],
                                         wqs_t[:, k * HID:(k + 1) * HID],
                                         x_t[:, 0:512], **st)
                        nc.tensor.matmul(u_ts[3][:],
                                         wqs_t[:, k * HID:(k + 1) * HID],
                                         x_t[:, 512:P2W], **st)

                    qnorms(2)
                    qnorms(3)
                    nc.gpsimd.dma_start(out=ssq_d[:], in_=ssq_sb[:])
                    score_half(1)

    lp.__exit__(None, None, None)
    nc.compile()
    return nc


def _host_prep(inputs):
    f32 = np.float32
    Wi, Wt = np.asarray(inputs["Wi"], f32), np.asarray(inputs["Wt"], f32)
    bi, bt = np.asarray(inputs["bi"], f32), np.asarray(inputs["bt"], f32)
    g1, b1 = np.asarray(inputs["g1"], f32), np.asarray(inputs["b1"], f32)
    g2, b2 = np.asarray(inputs["g2"], f32), np.asarray(inputs["b2"], f32)
    Wq, bq = np.asarray(inputs["Wq"], f32), np.asarray(inputs["bq"], f32)
    Wk, bk = np.asarray(inputs["Wk"], f32), np.asarray(inputs["bk"], f32)
    Wv, bv = np.asarray(inputs["Wv"], f32), np.asarray(inputs["bv"], f32)
    Wo, bo = np.asarray(inputs["Wo"], f32), np.asarray(inputs["bo"], f32)

    Wc = np.concatenate([Wi, Wt], axis=1)          # [128, 2816]
    bc = bi + bt
    Wc_c = Wc - Wc.mean(axis=0, keepdims=True)     # fold LN1 mean
    bc_c = bc - bc.mean()

    Wq_f = (Wq * g1[None, :]) * SCALE_INV
    bq_f = (bq + Wq @ b1) * SCALE_INV
    Wk_f = Wk * g1[None, :]
    bk_f = bk + Wk @ b1
    Wv_f = Wv * g1[None, :]
    bv_f = bv + Wv @ b1
    Wv_c = Wv_f - Wv_f.mean(axis=0, keepdims=True)  # fold LN2 mean
    bv_c = bv_f - bv_f.mean()
    Wo_f = Wo * g2[None, :]
    bo_f = bo + Wo @ b2

    # fast path requires the value/output biases to vanish (true here: all
    # reference biases are zeros) -- cosine invariance then absorbs the LN
    # scales on the query path and LN2 on support.
    assert np.abs(bv_c).max() < 1e-6 and np.abs(bo_f).max() < 1e-6, \
        "nonzero bv/bo: collapsed query path would be inexact"

    Wov = Wo_f @ Wv_c                              # [128, 128]
    Wqs = Wov @ Wc_c                               # [128, 2816]
    ub = Wov @ bc_c                                # [128]

    def chunked(w):      # [2816, 128] -> [128 part, KT*128] f16
        return np.ascontiguousarray(
            w.T.reshape(KT, 128, HID).transpose(1, 0, 2).reshape(128, KT * HID)
        ).astype(np.float16)

    blk = np.arange(SROWS) // S
    mask = np.where(blk[:, None] == blk[None, :], 0.0, -1e30).astype(f32)

    wqkvo = np.concatenate([Wq_f.T, Wk_f.T, Wv_c.T, Wo_f.T], axis=1)
    biases = np.stack([bc_c, bq_f, bk_f, ub, np.ones(HID, f32),
                       np.zeros(HID, f32), np.zeros(HID, f32),
                       np.zeros(HID, f32)], axis=1)
    common = {
        "ws": chunked(Wc_c),
        "wqs": chunked(Wqs),
        "wqkvo": np.ascontiguousarray(wqkvo),
        "biases": np.ascontiguousarray(biases),
        "onesr": np.ones((1, HID), f32),
        "mask": mask, "ident": np.eye(128, dtype=f32),
    }

    si = np.asarray(inputs["support_images"], f32)
    st = np.asarray(inputs["support_texts"], f32)
    qi = np.asarray(inputs["query_images"], f32)
    qt = np.asarray(inputs["query_texts"], f32)

    in_maps = []
    for m in range(NCORES):
        ts = slice(m * TPC, (m + 1) * TPC)
        Xq = np.concatenate([qi[ts].reshape(QROWS, DI),
                             qt[ts].reshape(QROWS, DTXT)], axis=1)
        Xs = np.concatenate([si[ts].reshape(SROWS, DI),
                             st[ts].reshape(SROWS, DTXT)], axis=1)
        X = np.concatenate([Xs, Xq], axis=0)        # [2176, 2816] support first
        xT = np.ascontiguousarray(X.T.astype(np.float16))  # [2816, 2176] f16
        in_maps.append({"xT": xT, **common})
    return in_maps


def _run(in_maps, trace=False, **kw):
    from concourse.bass_utils import run_bass_kernel_spmd
    global _prog
    if _prog is None:
        _prog = _build()
    return run_bass_kernel_spmd(_prog, in_maps, list(range(NCORES)),
                                trace=trace, **kw)


def _finish(res):
    """Host-side cosine normalization: logits = 10*U/(|qf| |sf|), with the
    reference's 1e-8 norm clips applied exactly."""
    outs = []
    for m in range(NCORES):
        u = res.results[m]["uraw"]                       # [TPC, Q, S]
        qn = np.sqrt(res.results[m]["ssq"].reshape(TPC, Q))
        sn = np.sqrt(res.results[m]["sss"].reshape(TPC, S))
        qn = np.maximum(qn, 1e-8)
        sn = np.maximum(sn, 1e-8)
        outs.append(10.0 * u / (qn[:, :, None] * sn[:, None, :]))
    return np.concatenate(outs, axis=0).astype(np.float32)


def kernel(**inputs) -> np.ndarray:
    in_maps = _host_prep(inputs)
    res = _run(in_maps)
    return _finish(res)


# revision 14
# speedup vs baseline: 2.9305x; 1.1624x over previous
"""Trainium2 Bass kernel for nn_MetaLearner (meta-learning attention + cosine
prototype scoring), data-parallel over tasks on 8 NeuronCores.

Math (per task):
  c   = [img, txt] @ Wc.T + bc                (Wc = concat(Wi, Wt))
  h   = LN1(c);  q,k,v = h @ W{q,k,v}.T + b   (queries: seqlen=1 -> ctx = v)
  ctx = softmax(q k^T / sqrt(128)) v          (support: seqlen=4)
  f   = LN2(ctx) @ Wo.T + bo
  logits[t,q,c] = 10 * cos(qf[t,q], sf[t,c])

Key folds (exact for this problem's parameters; asserted on host):
  - LN gains/biases folded into following projections; LN mean-subtraction
    folded into producing weights (column-centered weights give zero-mean
    projections, preserved through attention since softmax rows sum to 1).
  - With centered weights LN is a pure per-column scale and cosine similarity
    is scale-invariant, so the QUERY path (seqlen 1, zero bv/bo) collapses:
        cos(Wo.LN2(Wv.LN1(c)), p) == cos((Wo@Wv@Wc).x, p)
    One composed-weight GEMM streams all query work; support keeps full
    attention but its LN2 is likewise absorbed.
  - Cosine divisions happen on the host: the device ships raw dot products
    U = qf_raw^T sf_raw plus squared norms; kernel() divides after the
    gather.  No rsqrt/reciprocal on device; the scalar engine needs only
    {identity, square, ln, exp, copy}: 3 activation-table loads total.
  - 1/sqrt(128) folded into Wq.  Inputs stream as float16; f32 PSUM.
  - The support block and both weight matrices are pre-chunked on host into
    [128, KT*128] so every DMA is a cheap contiguous 2D transfer; the
    support pass runs FIRST so the long attention chain hides under the
    query stream.
"""
import sys
sys.path.insert(0, "/opt/trn_rl_repo")
import numpy as np

HID = 128
T, Q, S = 256, 64, 4
DI, DTXT = 2048, 768
NCORES = 8
TPC = T // NCORES               # 32 tasks per core
FEAT = DI + DTXT                # 2816
KT = FEAT // 128                # 22 contraction chunks
QROWS = TPC * Q                 # 2048 query rows per core
SROWS = TPC * S                 # 128 support rows per core
SCALE_INV = 1.0 / (np.sqrt(HID) + 1e-8)
EPS = 1e-5

_prog = None  # cached compiled Bass program


def _build():
    import concourse.bacc as bacc
    import concourse.tile as tile
    import concourse.mybir as mybir
    import concourse.bass as _b

    F32 = mybir.dt.float32
    F32R = mybir.dt.float32r
    F16 = mybir.dt.float16
    AFT = mybir.ActivationFunctionType
    AX = mybir.AxisListType
    ALU = mybir.AluOpType

    nc = bacc.Bacc()
    xq_d = nc.declare_dram_parameter("xq", [FEAT, QROWS], F16, isOutput=False)
    xs_d = nc.declare_dram_parameter("xs", [128, KT * SROWS], F16,
                                     isOutput=False)
    ws_d = nc.declare_dram_parameter("ws", [128, KT * HID], F16, isOutput=False)
    wqs_d = nc.declare_dram_parameter("wqs", [128, KT * HID], F16,
                                      isOutput=False)
    wqkvo_d = nc.declare_dram_parameter("wqkvo", [HID, 4 * HID], F32R,
                                        isOutput=False)
    bias_d = nc.declare_dram_parameter("biases", [HID, 8], F32, isOutput=False)
    onesr_d = nc.declare_dram_parameter("onesr", [1, HID], F32R, isOutput=False)
    mask_d = nc.declare_dram_parameter("mask", [SROWS, SROWS], F32, isOutput=False)
    id_d = nc.declare_dram_parameter("ident", [128, 128], F32, isOutput=False)
    u_d = nc.declare_dram_parameter("uraw", [TPC, Q, S], F32, isOutput=True)
    ssq_d = nc.declare_dram_parameter("ssq", [1, QROWS], F32, isOutput=True)
    sss_d = nc.declare_dram_parameter("sss", [1, SROWS], F32, isOutput=True)

    lp = nc.allow_low_precision(reason="f16 stream / f32r tail matmuls")
    lp.__enter__()

    with tile.TileContext(nc) as tc:
        with (
            tc.tile_pool(name="wts", bufs=1) as wts,
            tc.tile_pool(name="qfp", bufs=1) as qfp,
            tc.tile_pool(name="xap", bufs=KT) as xap,
            tc.tile_pool(name="xbp", bufs=KT) as xbp,
            tc.tile_pool(name="wk", bufs=2) as wk,
        ):
            ws_t = wts.tile([128, KT * HID], F16)
            wqs_t = wts.tile([128, KT * HID], F16)
            xs_t = wts.tile([128, KT * SROWS], F16)
            wqkvo_t = wts.tile([128, 4 * HID], F32R)
            bias_t = wts.tile([HID, 8], F32)
            ones_c = wts.tile([128, 1], F32R)
            ones_r = wts.tile([1, 128], F32R)
            mask_t = wts.tile([SROWS, SROWS], F32)
            id_t = wts.tile([128, 128], F32)
            eps_t = wts.tile([1, 1], F32)

            xa_tiles, xb_tiles = [], []

            def _xa(k):
                t = xap.tile([128, 1024], F16, tag="xa", name=f"xa{k}")
                eng = nc.sync if k % 2 == 0 else nc.gpsimd
                eng.dma_start(out=t, in_=xq_d[k * 128:(k + 1) * 128, 0:1024])
                xa_tiles.append(t)

            def _xb(k):
                t = xbp.tile([128, 1024], F16, tag="xb", name=f"xb{k}")
                eng = nc.sync if k % 2 == 0 else nc.gpsimd
                eng.dma_start(out=t,
                              in_=xq_d[k * 128:(k + 1) * 128, 1024:QROWS])
                xb_tiles.append(t)

            # prelude DMAs.  sync queue: support block + small tables + even
            # query chunks; gpsimd queue: weights + odd query chunks.
            nc.sync.dma_start(out=xs_t, in_=xs_d[:])
            nc.gpsimd.dma_start(out=ws_t, in_=ws_d[:])
            nc.sync.dma_start(out=wqkvo_t, in_=wqkvo_d[:])
            nc.sync.dma_start(out=bias_t, in_=bias_d[:])
            nc.sync.dma_start(out=ones_r, in_=onesr_d[:])
            nc.sync.dma_start(out=mask_t, in_=mask_d[:])
            nc.sync.dma_start(out=id_t, in_=id_d[:])
            nc.gpsimd.dma_start(out=wqs_t, in_=wqs_d[:])
            nc.gpsimd.dma_start(out=ones_c, in_=bias_d[:, 4:5])  # f32r cast
            for k in range(KT):
                _xa(k)
            for k in range(KT):
                _xb(k)
            nc.vector.memset(eps_t, EPS)

            bc_t = bias_t[:, 0:1]
            bq_t = bias_t[:, 1:2]
            bk_t = bias_t[:, 2:3]
            ub_t = bias_t[:, 3:4]
            wq_t = wqkvo_t[:, 0 * HID:1 * HID]
            wk_t = wqkvo_t[:, 1 * HID:2 * HID]
            wv_t = wqkvo_t[:, 2 * HID:3 * HID]
            wo_t = wqkvo_t[:, 3 * HID:4 * HID]

            # raw (unnormalized) features, f32r for the scoring matmuls
            qraw = [qfp.tile([128, 512], F32R, tag=f"qr{j}", name=f"qr{j}")
                    for j in range(4)]
            sraw = qfp.tile([128, SROWS], F32R, tag="sr")
            ssq_sb = qfp.tile([1, QROWS], F32, tag="ssqsb")
            sss_sb = qfp.tile([1, SROWS], F32, tag="ssssb")

            with tc.tile_pool(name="pu", bufs=1, space="PSUM") as pu:
                u_ts = [pu.tile([128, 512], F32, tag=f"u{j}", name=f"u{j}")
                        for j in range(4)]

                # ---- pass 0: support columns (pre-chunked block) ----
                with tc.tile_pool(name="psA", bufs=1, space="PSUM") as psA:
                    c_s = psA.tile([128, SROWS], F32, tag="cs")
                    for k in range(KT):
                        nc.tensor.matmul(
                            c_s[:], ws_t[:, k * HID:(k + 1) * HID],
                            xs_t[:, k * SROWS:(k + 1) * SROWS],
                            start=(k == 0), stop=(k == KT - 1))
                    cs_f = wk.tile([128, SROWS], F32, tag="csf")
                    nc.scalar.activation(out=cs_f, in_=c_s, func=AFT.Identity,
                                         bias=bc_t, scale=1.0)

                def qchunk(j, k, u):
                    x_t = (xa_tiles if j < 2 else xb_tiles)[k]
                    nc.tensor.matmul(
                        u[:], wqs_t[:, k * HID:(k + 1) * HID],
                        x_t[:, 512 * (j % 2):512 * (j % 2) + 512],
                        start=(k == 0), stop=(k == KT - 1))

                # ---- pass 1 (q0+q1), first few chunks ----
                for k in range(0, 4):
                    qchunk(0, k, u_ts[0])
                    qchunk(1, k, u_ts[1])

                with tc.tile_pool(name="pst", bufs=1, space="PSUM") as pst:
                    # ---- support tail (full attention on 128 columns) ----
                    sqs_ = wk.tile([128, SROWS], F32R, tag="sqs")
                    nc.scalar.activation(out=sqs_, in_=cs_f, func=AFT.Square,
                                         bias=0.0, scale=1.0)
                    ss1 = pst.tile([1, 512], F32, tag="ssps", bufs=1)
                    nc.tensor.matmul(ss1[:, :SROWS], ones_c[:], sqs_[:],
                                     start=True, stop=True)
                    ln_r = wk.tile([1, SROWS], F32, tag="lnr")
                    nc.scalar.activation(out=ln_r, in_=ss1[:, :SROWS],
                                         func=AFT.Ln, bias=eps_t[:],
                                         scale=1.0 / HID)
                    ir = wk.tile([1, SROWS], F32R, tag="rsr")
                    nc.scalar.activation(out=ir, in_=ln_r, func=AFT.Exp,
                                         bias=0.0, scale=-0.5)
                    R1 = pst.tile([128, SROWS], F32, tag="rps", bufs=1)
                    nc.tensor.matmul(R1[:], ones_r[:], ir[:],
                                     start=True, stop=True)
                    h_t = wk.tile([128, SROWS], F32R, tag="h")
                    nc.vector.tensor_mul(out=h_t, in0=cs_f, in1=R1[:])

                    q_ps = pst.tile([128, SROWS], F32, tag="pps", bufs=1)
                    nc.tensor.matmul(q_ps[:], wq_t, h_t[:], start=True, stop=True)
                    qT = wk.tile([128, SROWS], F32R, tag="qT")
                    nc.scalar.activation(out=qT, in_=q_ps, func=AFT.Identity,
                                         bias=bq_t, scale=1.0)
                    k_ps = pst.tile([128, SROWS], F32, tag="pps", bufs=1)
                    nc.tensor.matmul(k_ps[:], wk_t, h_t[:], start=True, stop=True)
                    kT = wk.tile([128, SROWS], F32R, tag="kT")
                    nc.scalar.activation(out=kT, in_=k_ps, func=AFT.Identity,
                                         bias=bk_t, scale=1.0)
                    # v in natural [rows, hid] layout: lhsT=h (K=hid, M=rows)
                    vn_ps = pst.tile([128, SROWS], F32, tag="pps", bufs=1)
                    nc.tensor.matmul(vn_ps[:], h_t[:], wv_t, start=True, stop=True)
                    vn = wk.tile([SROWS, HID], F32R, tag="vn")
                    nc.vector.tensor_copy(out=vn, in_=vn_ps)

                    s_ps = pst.tile([SROWS, SROWS], F32, tag="pps", bufs=1)
                    nc.tensor.matmul(s_ps[:], qT[:], kT[:], start=True, stop=True)
                    s_f = wk.tile([SROWS, SROWS], F32, tag="sf_")
                    nc.vector.tensor_add(out=s_f, in0=s_ps, in1=mask_t)
                    nmx = wk.tile([SROWS, 1], F32, tag="nmx")
                    nc.vector.tensor_reduce(out=nmx, in_=s_f, axis=AX.X,
                                            op=ALU.max, negate=True)
                    a_f = wk.tile([SROWS, SROWS], F32, tag="af")
                    asum = wk.tile([SROWS, 1], F32, tag="asum")
                    nc.scalar.activation(out=a_f, in_=s_f, func=AFT.Exp,
                                         bias=nmx, scale=1.0, accum_out=asum)
                    rs = wk.tile([SROWS, 1], F32, tag="rs")
                    nc.vector.reciprocal(out=rs, in_=asum)
                    nc.vector.tensor_scalar_mul(out=a_f, in0=a_f, scalar1=rs)

                    aT_ps = pst.tile([SROWS, SROWS], F32, tag="pps", bufs=1)
                    nc.tensor.matmul(aT_ps[:], a_f[:], id_t[:], is_transpose=True)
                    aT = wk.tile([SROWS, SROWS], F32R, tag="aT")
                    nc.vector.tensor_copy(out=aT, in_=aT_ps)
                    ctx_ps = pst.tile([128, SROWS], F32, tag="pps", bufs=1)
                    nc.tensor.matmul(ctx_ps[:], vn[:], aT[:], start=True, stop=True)
                    ctx_f = wk.tile([128, SROWS], F32R, tag="ctxf")
                    nc.vector.tensor_copy(out=ctx_f, in_=ctx_ps)
                    o_ps = pst.tile([128, SROWS], F32, tag="pps", bufs=1)
                    nc.tensor.matmul(o_ps[:], wo_t, ctx_f[:], start=True, stop=True)
                    nc.scalar.activation(out=sraw, in_=o_ps, func=AFT.Copy,
                                         scale=1.0)
                    sqo = wk.tile([128, SROWS], F32R, tag="sqo")
                    nc.vector.tensor_mul(out=sqo, in0=sraw, in1=sraw)
                    ss2 = pst.tile([1, 512], F32, tag="ssps", bufs=1)
                    nc.tensor.matmul(ss2[:, :SROWS], ones_c[:], sqo[:],
                                     start=True, stop=True)
                    nc.vector.tensor_copy(out=sss_sb, in_=ss2[:, :SROWS])
                    nc.gpsimd.dma_start(out=sss_d[:], in_=sss_sb[:])

                    # ---- pass 1 rest ----
                    for k in range(4, KT):
                        qchunk(0, k, u_ts[0])
                        qchunk(1, k, u_ts[1])

                    def qnorms(j):
                        """raw_j = u_j + ub; stage ||col||^2 for the host."""
                        nc.vector.tensor_scalar_add(out=qraw[j], in0=u_ts[j],
                                                    scalar1=ub_t)
                        sq = wk.tile([128, 512], F32R, tag="sq")
                        nc.vector.tensor_mul(out=sq, in0=qraw[j], in1=qraw[j])
                        ss = pst.tile([1, 512], F32, tag="ssps", bufs=1)
                        nc.tensor.matmul(ss[:], ones_c[:], sq[:],
                                         start=True, stop=True)
                        nc.vector.tensor_copy(
                            out=ssq_sb[:, 512 * j:512 * (j + 1)], in_=ss[:])

                    qnorms(0)
                    qnorms(1)

                    # ---- pass 2a (q2+q3) ----
                    for k in range(0, 14):
                        qchunk(2, k, u_ts[2])
                        qchunk(3, k, u_ts[3])

                    u_base = u_d[:]
                    usb_tiles = []

                    def score(ts0, n, U):
                        for i in range(n):
                            t = ts0 + i
                            j = t // 8
                            col = 64 * (t % 8)
                            nc.tensor.matmul(
                                U[0:64, 4 * (t % 16):4 * (t % 16) + 4],
                                qraw[j][:, col:col + 64],
                                sraw[:, 4 * t:4 * t + 4],
                                start=True, stop=True)

                    def flush_half(hf, U):
                        U_sb = wk.tile([64, 64], F32, tag="usb",
                                       name=f"usb{hf}")
                        nc.vector.tensor_copy(out=U_sb, in_=U)
                        dst = _b.AP(tensor=u_base.tensor,
                                    offset=u_base.offset + 4096 * hf,
                                    ap=[[4, 64], [256, 16], [1, 4]])
                        nc.sync.dma_start(
                            out=dst, in_=U_sb.rearrange("p (g b) -> p g b", b=4))

                    U0 = pst.tile([64, 64], F32, tag="ups", bufs=1, name="ups0")
                    score(0, 16, U0)
                    flush_half(0, U0)

                    # ---- pass 2b: finish u2 first, then u3 ----
                    for k in range(14, KT):
                        qchunk(2, k, u_ts[2])
                    qnorms(2)
                    U1 = pst.tile([64, 64], F32, tag="ups", bufs=1, name="ups1")
                    score(16, 8, U1)
                    for k in range(14, KT):
                        qchunk(3, k, u_ts[3])
                    qnorms(3)
                    score(24, 8, U1)
                    flush_half(1, U1)
                    nc.gpsimd.dma_start(out=ssq_d[:], in_=ssq_sb[:])

    lp.__exit__(None, None, None)
    nc.compile()
    return nc


def _host_prep(inputs):
    f32 = np.float32
    Wi, Wt = np.asarray(inputs["Wi"], f32), np.asarray(inputs["Wt"], f32)
    bi, bt = np.asarray(inputs["bi"], f32), np.asarray(inputs["bt"], f32)
    g1, b1 = np.asarray(inputs["g1"], f32), np.asarray(inputs["b1"], f32)
    g2, b2 = np.asarray(inputs["g2"], f32), np.asarray(inputs["b2"], f32)
    Wq, bq = np.asarray(inputs["Wq"], f32), np.asarray(inputs["bq"], f32)
    Wk, bk = np.asarray(inputs["Wk"], f32), np.asarray(inputs["bk"], f32)
    Wv, bv = np.asarray(inputs["Wv"], f32), np.asarray(inputs["bv"], f32)
    Wo, bo = np.asarray(inputs["Wo"], f32), np.asarray(inputs["bo"], f32)

    Wc = np.concatenate([Wi, Wt], axis=1)          # [128, 2816]
    bc = bi + bt
    Wc_c = Wc - Wc.mean(axis=0, keepdims=True)     # fold LN1 mean
    bc_c = bc - bc.mean()

    Wq_f = (Wq * g1[None, :]) * SCALE_INV
    bq_f = (bq + Wq @ b1) * SCALE_INV
    Wk_f = Wk * g1[None, :]
    bk_f = bk + Wk @ b1
    Wv_f = Wv * g1[None, :]
    bv_f = bv + Wv @ b1
    Wv_c = Wv_f - Wv_f.mean(axis=0, keepdims=True)  # fold LN2 mean
    bv_c = bv_f - bv_f.mean()
    Wo_f = Wo * g2[None, :]
    bo_f = bo + Wo @ b2

    # fast path requires the value/output biases to vanish (true here: all
    # reference biases are zeros) -- cosine invariance then absorbs the LN
    # scales on the query path and LN2 on support.
    assert np.abs(bv_c).max() < 1e-6 and np.abs(bo_f).max() < 1e-6, \
        "nonzero bv/bo: collapsed query path would be inexact"

    Wov = Wo_f @ Wv_c                              # [128, 128]
    Wqs = Wov @ Wc_c                               # [128, 2816]
    ub = Wov @ bc_c                                # [128]

    def chunked(wT, ncol):   # [2816, ncol] -> [128 part, KT*ncol] f16
        return np.ascontiguousarray(
            wT.reshape(KT, 128, ncol).transpose(1, 0, 2).reshape(128, KT * ncol)
        ).astype(np.float16)

    blk = np.arange(SROWS) // S
    mask = np.where(blk[:, None] == blk[None, :], 0.0, -1e30).astype(f32)

    wqkvo = np.concatenate([Wq_f.T, Wk_f.T, Wv_c.T, Wo_f.T], axis=1)
    biases = np.stack([bc_c, bq_f, bk_f, ub, np.ones(HID, f32),
                       np.zeros(HID, f32), np.zeros(HID, f32),
                       np.zeros(HID, f32)], axis=1)
    common = {
        "ws": chunked(Wc_c.T, HID),
        "wqs": chunked(Wqs.T, HID),
        "wqkvo": np.ascontiguousarray(wqkvo),
        "biases": np.ascontiguousarray(biases),
        "onesr": np.ones((1, HID), f32),
        "mask": mask, "ident": np.eye(128, dtype=f32),
    }

    si = np.asarray(inputs["support_images"], f32)
    st = np.asarray(inputs["support_texts"], f32)
    qi = np.asarray(inputs["query_images"], f32)
    qt = np.asarray(inputs["query_texts"], f32)

    in_maps = []
    for m in range(NCORES):
        ts = slice(m * TPC, (m + 1) * TPC)
        Xq = np.concatenate([qi[ts].reshape(QROWS, DI),
                             qt[ts].reshape(QROWS, DTXT)], axis=1)
        Xs = np.concatenate([si[ts].reshape(SROWS, DI),
                             st[ts].reshape(SROWS, DTXT)], axis=1)
        xq = np.ascontiguousarray(Xq.T.astype(np.float16))  # [2816, 2048]
        xs = chunked(np.ascontiguousarray(Xs.T), SROWS)     # [128, KT*128]
        in_maps.append({"xq": xq, "xs": xs, **common})
    return in_maps


def _run(in_maps, trace=False, **kw):
    from concourse.bass_utils import run_bass_kernel_spmd
    global _prog
    if _prog is None:
        _prog = _build()
    return run_bass_kernel_spmd(_prog, in_maps, list(range(NCORES)),
                                trace=trace, **kw)


def _finish(res):
    """Host-side cosine normalization: logits = 10*U/(|qf| |sf|), with the
    reference's 1e-8 norm clips applied exactly."""
    outs = []
    for m in range(NCORES):
        u = res.results[m]["uraw"]                       # [TPC, Q, S]
        qn = np.sqrt(res.results[m]["ssq"].reshape(TPC, Q))
        sn = np.sqrt(res.results[m]["sss"].reshape(TPC, S))
        qn = np.maximum(qn, 1e-8)
        sn = np.maximum(sn, 1e-8)
        outs.append(10.0 * u / (qn[:, :, None] * sn[:, None, :]))
    return np.concatenate(outs, axis=0).astype(np.float32)


def kernel(**inputs) -> np.ndarray:
    in_maps = _host_prep(inputs)
    res = _run(in_maps)
    return _finish(res)
